# revision 22
# baseline (speedup 1.0000x reference)
"""VQ codebook-lookup kernel for 8 Trainium2 NeuronCores.

Problem: z [8, 4096, 32] f32, codebook [8192, 32] f32.
  dist[n,k] = ||z_n||^2 + ||e_k||^2 - 2 z_n.e_k
  idx = argmin_k dist  (first index on fp32 ties, like jnp.argmin)
  z_q = codebook[idx]; z_q_st = z + (z_q - z); loss = 1.25*mean((z_q-z)^2)

Sharding: data-parallel over the batch axis -- core i handles z[i]
(4096 tokens), codebook replicated. No collectives; the host sums the 8
scalar loss partials and concatenates the shard outputs.

Numerics (matches the fp32 reference's argmin ties):
  * ||e_k||^2 rounds away entirely in fp32 (|e|<=1/8192 while ||z||^2~32),
    so dist == fl(||z||^2 - 2c) with c = fl(z.e) -- verified bitwise.
  * fl(A - 2c) quantizes the k-dependent part to multiples of u=ulp32(A),
    creating ~200 argmin ties per batch that jnp.argmin breaks by first
    index.  The kernel reproduces the same buckets with ONE deterministic
    rounding: t = rne_int16(2c * (1/u)) on the scalar engine (the scale is
    a power of two, so the affine is exact; a per-token integer offset
    cannot change the argmax).  PE computes raw 2c only (tiny partials,
    so its internal accumulation-tree order contributes ~2^-32 noise --
    augmenting A into the matmul instead was observed to produce +-1 ulp
    scatter at 10% rate on real HW and ~106 wrong indices).
  * max + first-index via int16 tensor_tensor max tree (2x packed mode)
    and the DVE MaxIndex instruction (first-match semantics, HW-verified).

Engines: PE raw-2c matmuls with 4-way tile_position row tiling (contraction
is 32, so four token tiles stream through disjoint 32-row PE bands
concurrently; bit-exactness of row-tiled fp32 matmul was verified on HW).
ACT evacuates PSUM with the fused int16 quantization.  DVE owns the
max-tree + index match.  GPSIMD does the codebook row gather (indirect
DMA), the straight-through output, and the loss partial.
"""

import os
from contextlib import ExitStack

import numpy as np

import concourse.bass as bass
import concourse.bacc as bacc
import concourse.mybir as mybir
import concourse.tile as tile
from concourse.bass_utils import run_bass_kernel_spmd

B, T, D = 8, 4096, 32
K = 8192
P = 128
NT = T // P           # 32 token tiles per core
NG = NT // 4          # 8 groups of 4 band-parallel token tiles
CHUNK = 1024          # ACT evacuation chunk (2 psum banks) per band
DT = mybir.dt

_NC = None


def build_nc() -> bass.Bass:
    nc = bacc.Bacc()

    z4T = nc.declare_dram_parameter("z4T", [P, T], DT.float32, isOutput=False)
    cb4T = nc.declare_dram_parameter("cb4T", [P, K], DT.float32, isOutput=False)
    ascale = nc.declare_dram_parameter("ascale", [P, NT], DT.float32, isOutput=False)
    zre = nc.declare_dram_parameter("zre", [P, NT * D], DT.float32, isOutput=False)
    cbtab = nc.declare_dram_parameter("cbtab", [K, D], DT.float32, isOutput=False)

    idx_out = nc.declare_dram_parameter("idx_out", [P, NT], DT.uint32, isOutput=True)
    zqst_out = nc.declare_dram_parameter("zqst_out", [P, NT * D], DT.float32, isOutput=True)
    sq_out = nc.declare_dram_parameter("sq_out", [P, NT], DT.float32, isOutput=True)

    with tile.TileContext(nc) as tc, ExitStack() as ctx:
        const_pool = ctx.enter_context(tc.tile_pool(name="const", bufs=1))
        psum_pool = ctx.enter_context(tc.tile_pool(name="psum", bufs=2, space="PSUM"))
        t16_pool = ctx.enter_context(tc.tile_pool(name="t16", bufs=5))
        tree_pool = ctx.enter_context(tc.tile_pool(name="tree", bufs=2))
        small_pool = ctx.enter_context(tc.tile_pool(name="small", bufs=2))

        # ---- constants, loaded once ----
        # Chain the two matmul-feeding loads so the first LDWEIGHTS only
        # needs a single semaphore wait (the S3_LW struct allows one).
        from concourse.tile import add_dep_helper

        # Chain the matmul operand loads onto one semaphore stream (every
        # matmul then needs only one wait -- the S3_LW limit), with the
        # codebook split head/tail so the PE can start before the full
        # 4 MB codebook replica lands.
        CH0 = K // 2        # head codebook columns
        z4T_sb = const_pool.tile([P, T], DT.float32)
        d1 = nc.sync.dma_start(z4T_sb[:], z4T[:])
        cb4h_sb = const_pool.tile([P, CH0], DT.float32)
        d2 = nc.sync.dma_start(cb4h_sb[:], cb4T[:, :CH0])
        cb4t_sb = const_pool.tile([P, K - CH0], DT.float32)
        d3 = nc.sync.dma_start(cb4t_sb[:], cb4T[:, CH0:])
        add_dep_helper(d1.ins, d2.ins, True, "chain const loads")
        add_dep_helper(d2.ins, d3.ins, True, "chain const loads")

        def z4_slice(r, t):
            return z4T_sb[32 * r:32 * r + 32, t * P:(t + 1) * P]

        def cb4_slice(r, c0):
            if c0 < CH0:
                return cb4h_sb[32 * r:32 * r + 32, c0:c0 + 512]
            return cb4t_sb[32 * r:32 * r + 32, c0 - CH0:c0 - CH0 + 512]

        ascale_sb = const_pool.tile([P, NT], DT.float32)
        nc.sync.dma_start(ascale_sb[:], ascale[:])
        zre_sb = const_pool.tile([P, NT * D], DT.float32)
        nc.sync.dma_start(zre_sb[:], zre[:])

        idx_all = const_pool.tile([P, NT], DT.uint32)
        sq_all = const_pool.tile([P, NT], DT.float32)

        for g in range(NG):
            tiles = [g * 4 + r for r in range(4)]
            t16s = [t16_pool.tile([P, K], DT.int16, tag="t16", name=f"t16_g{g}_{r}") for r in range(4)]

            # PE: token tiles stream through disjoint 32-row PE bands.
            # Pair-major order (tiles r,r+1 interleaved per chunk, two
            # shared double-buffered PSUM tags) lets each pair's t16
            # complete halfway through the group, so the DVE max/index
            # stage trails the PE by ~2 tiles instead of a full group.
            order = [(ch, r) for pair in (0, 2) for ch in range(K // CHUNK)
                     for r in (pair, pair + 1)]
            for ch, r in order:
                t = tiles[r]
                ptile = psum_pool.tile([P, CHUNK], DT.float32, tag=f"ps{r % 2}", name=f"ps_{g}_{ch}_{r}")
                for j in range(CHUNK // 512):
                    c0 = ch * CHUNK + j * 512
                    nc.tensor.matmul(
                        ptile[:, j * 512:(j + 1) * 512],
                        z4_slice(r, t),
                        cb4_slice(r, c0),
                        start=True,
                        stop=True,
                        tile_position=(32 * r, 0),
                    )

                nc.scalar.activation(
                    t16s[r][:, ch * CHUNK:(ch + 1) * CHUNK],
                    ptile[:],
                    mybir.ActivationFunctionType.Identity,
                    bias=0.0,
                    scale=ascale_sb[:, t:t + 1],
                )

            for r in range(4):
                t = tiles[r]
                t16 = t16s[r]

                # int16 pairwise max tree (tensor_tensor runs 2x on packed
                # 16-bit operands)
                tr1 = tree_pool.tile([P, K // 2], DT.int16)
                nc.vector.tensor_tensor(tr1[:], t16[:, :K // 2], t16[:, K // 2:], op=mybir.AluOpType.max)
                tr2 = tree_pool.tile([P, K // 4], DT.int16)
                nc.vector.tensor_tensor(tr2[:], tr1[:, :K // 4], tr1[:, K // 4:], op=mybir.AluOpType.max)
                tr3 = tree_pool.tile([P, K // 8], DT.int16)
                nc.vector.tensor_tensor(tr3[:], tr2[:, :K // 8], tr2[:, K // 8:], op=mybir.AluOpType.max)
                tr4 = tree_pool.tile([P, K // 16], DT.int16)
                nc.vector.tensor_tensor(tr4[:], tr3[:, :K // 16], tr3[:, K // 16:], op=mybir.AluOpType.max)
                gmax8 = small_pool.tile([P, 8], DT.int16, tag="gmax8")
                nc.vector.max(gmax8[:], tr4[:])

                # first index where t16 == gmax (ties resolve to lowest k,
                # matching jnp.argmin); gmax8[:, 0] is the row max.
                idx8 = small_pool.tile([P, 8], DT.uint16, tag="idx8")
                nc.vector.max_index(idx8[:], gmax8[:], t16[:])
                nc.vector.tensor_copy(idx_all[:, t:t + 1], idx8[:, 0:1])

                # z_q gather: codebook row per token
                zq = small_pool.tile([P, D], DT.float32, tag="zq")
                nc.gpsimd.indirect_dma_start(
                    out=zq[:],
                    out_offset=None,
                    in_=cbtab[:],
                    in_offset=bass.IndirectOffsetOnAxis(ap=idx_all[:, t:t + 1], axis=0),
                )

                zslice = zre_sb[:, t * D:(t + 1) * D]
                d1 = small_pool.tile([P, D], DT.float32, tag="d1")
                nc.gpsimd.tensor_tensor(d1[:], zq[:], zslice, op=mybir.AluOpType.subtract)
                zqst = small_pool.tile([P, D], DT.float32, tag="zqst")
                nc.gpsimd.tensor_tensor(zqst[:], zslice, d1[:], op=mybir.AluOpType.add)
                nc.sync.dma_start(zqst_out[:, t * D:(t + 1) * D], zqst[:])

                # loss partial: sum(d1^2) per partition
                junk = small_pool.tile([P, D], DT.float32, tag="junk")
                nc.vector.scalar_tensor_tensor(
                    junk[:],
                    d1[:],
                    1.0,
                    d1[:],
                    op0=mybir.AluOpType.mult,
                    op1=mybir.AluOpType.mult,
                    accum_out=sq_all[:, t:t + 1],
                )

        nc.sync.dma_start(idx_out[:], idx_all[:])
        nc.sync.dma_start(sq_out[:], sq_all[:])

    nc.finalize()
    return nc


def _get_nc() -> bass.Bass:
    global _NC
    if _NC is None:
        _NC = build_nc()
    return _NC


def _prep_core_inputs(z_core: np.ndarray, cb: np.ndarray) -> dict[str, np.ndarray]:
    z_core = np.ascontiguousarray(z_core, dtype=np.float32)   # [T, D]
    A = np.sum(z_core * z_core, axis=1, dtype=np.float32)     # [T]

    z4T = np.tile((2.0 * z_core).T, (4, 1))                   # [128, T]
    cb4T = np.tile(cb.T, (4, 1))                              # [128, K]

    # quantization scale: 1/ulp32(A) = 2^(24 - ex), a power of two.
    _, ex = np.frexp(A.astype(np.float64))
    inv_u = np.exp2(24.0 - ex).astype(np.float32)
    ascale = np.ascontiguousarray(inv_u.reshape(NT, P).T)     # [P, NT]

    zre = np.ascontiguousarray(
        z_core.reshape(NT, P, D).transpose(1, 0, 2).reshape(P, NT * D)
    )
    return {
        "z4T": np.ascontiguousarray(z4T),
        "cb4T": np.ascontiguousarray(cb4T),
        "ascale": ascale,
        "zre": zre,
        "cbtab": np.ascontiguousarray(cb, dtype=np.float32),
    }


def kernel(z: np.ndarray, codebook: np.ndarray):
    z = np.asarray(z, dtype=np.float32)
    cb = np.asarray(codebook, dtype=np.float32)
    assert z.shape == (B, T, D) and cb.shape == (K, D)

    nc = _get_nc()
    in_maps = [_prep_core_inputs(z[i], cb) for i in range(B)]
    trace = bool(int(os.environ.get("VQ_TRACE", "0")))
    res = run_bass_kernel_spmd(nc, in_maps, list(range(B)), trace=trace)
    if trace and res.exec_time_ns is not None:
        print(f"HW exec time: {res.exec_time_ns} ns")

    idx_full = np.empty((B, T), np.int64)
    zqst_full = np.empty((B, T, D), np.float32)
    total_sq = 0.0
    for i in range(B):
        out = res.results[i]
        idx_core = out["idx_out"].astype(np.int64)            # [P, NT]
        idx_full[i] = idx_core.T.reshape(T)                   # token n = t*128+p
        zqst_full[i] = (
            out["zqst_out"].reshape(P, NT, D).transpose(1, 0, 2).reshape(T, D)
        )
        total_sq += float(np.sum(out["sq_out"], dtype=np.float64))

    loss = np.float32(1.25 * total_sq / float(B * T * D))
    idx_out = idx_full.reshape(-1).astype(np.int32)
    return zqst_full, idx_out, loss


# revision 23
# speedup vs baseline: 1.0003x; 1.0003x over previous
"""VQ codebook-lookup kernel for 8 Trainium2 NeuronCores.

Problem: z [8, 4096, 32] f32, codebook [8192, 32] f32.
  dist[n,k] = ||z_n||^2 + ||e_k||^2 - 2 z_n.e_k
  idx = argmin_k dist  (first index on fp32 ties, like jnp.argmin)
  z_q = codebook[idx]; z_q_st = z + (z_q - z); loss = 1.25*mean((z_q-z)^2)

Sharding: data-parallel over the batch axis -- core i handles z[i]
(4096 tokens), codebook replicated. No collectives; the host sums the 8
scalar loss partials and concatenates the shard outputs.

Numerics (matches the fp32 reference's argmin ties):
  * ||e_k||^2 rounds away entirely in fp32 (|e|<=1/8192 while ||z||^2~32),
    so dist == fl(||z||^2 - 2c) with c = fl(z.e) -- verified bitwise.
  * fl(A - 2c) quantizes the k-dependent part to multiples of u=ulp32(A),
    creating ~200 argmin ties per batch that jnp.argmin breaks by first
    index.  The kernel reproduces the same buckets with ONE deterministic
    rounding: t = rne_int16(2c * (1/u)) on the scalar engine (the scale is
    a power of two, so the affine is exact; a per-token integer offset
    cannot change the argmax).  PE computes raw 2c only (tiny partials,
    so its internal accumulation-tree order contributes ~2^-32 noise --
    augmenting A into the matmul instead was observed to produce +-1 ulp
    scatter at 10% rate on real HW and ~106 wrong indices).
  * max + first-index via int16 tensor_tensor max tree (2x packed mode)
    and the DVE MaxIndex instruction (first-match semantics, HW-verified).

Engines: PE raw-2c matmuls with 4-way tile_position row tiling (contraction
is 32, so four token tiles stream through disjoint 32-row PE bands
concurrently; bit-exactness of row-tiled fp32 matmul was verified on HW).
ACT evacuates PSUM with the fused int16 quantization.  DVE owns the
max-tree + index match.  GPSIMD does the codebook row gather (indirect
DMA), the straight-through output, and the loss partial.
"""

import os
from contextlib import ExitStack

import numpy as np

import concourse.bass as bass
import concourse.bacc as bacc
import concourse.mybir as mybir
import concourse.tile as tile
from concourse.bass_utils import run_bass_kernel_spmd

B, T, D = 8, 4096, 32
K = 8192
P = 128
NT = T // P           # 32 token tiles per core
NG = NT // 4          # 8 groups of 4 band-parallel token tiles
CHUNK = 1024          # ACT evacuation chunk (2 psum banks) per band
DT = mybir.dt

_NC = None


def build_nc() -> bass.Bass:
    nc = bacc.Bacc()

    z4T = nc.declare_dram_parameter("z4T", [P, T], DT.float32, isOutput=False)
    cb4T = nc.declare_dram_parameter("cb4T", [P, K], DT.float32, isOutput=False)
    ascale = nc.declare_dram_parameter("ascale", [P, NT], DT.float32, isOutput=False)
    zre = nc.declare_dram_parameter("zre", [P, NT * D], DT.float32, isOutput=False)
    cbtab = nc.declare_dram_parameter("cbtab", [K, D], DT.float32, isOutput=False)

    idx_out = nc.declare_dram_parameter("idx_out", [P, NT], DT.uint32, isOutput=True)
    zqst_out = nc.declare_dram_parameter("zqst_out", [P, NT * D], DT.float32, isOutput=True)
    sq_out = nc.declare_dram_parameter("sq_out", [P, NT], DT.float32, isOutput=True)

    with tile.TileContext(nc) as tc, ExitStack() as ctx:
        const_pool = ctx.enter_context(tc.tile_pool(name="const", bufs=1))
        psum_pool = ctx.enter_context(tc.tile_pool(name="psum", bufs=2, space="PSUM"))
        t16_pool = ctx.enter_context(tc.tile_pool(name="t16", bufs=5))
        tree_pool = ctx.enter_context(tc.tile_pool(name="tree", bufs=2))
        small_pool = ctx.enter_context(tc.tile_pool(name="small", bufs=2))

        # ---- constants, loaded once ----
        # Chain the two matmul-feeding loads so the first LDWEIGHTS only
        # needs a single semaphore wait (the S3_LW struct allows one).
        from concourse.tile import add_dep_helper

        # Chain the matmul operand loads onto one semaphore stream (every
        # matmul then needs only one wait -- the S3_LW limit), with the
        # codebook split head/tail so the PE can start before the full
        # 4 MB codebook replica lands.
        CH0 = K // 2        # head codebook columns
        z4T_sb = const_pool.tile([P, T], DT.float32)
        d1 = nc.sync.dma_start(z4T_sb[:], z4T[:])
        cb4h_sb = const_pool.tile([P, CH0], DT.float32)
        d2 = nc.sync.dma_start(cb4h_sb[:], cb4T[:, :CH0])
        cb4t_sb = const_pool.tile([P, K - CH0], DT.float32)
        d3 = nc.sync.dma_start(cb4t_sb[:], cb4T[:, CH0:])
        add_dep_helper(d1.ins, d2.ins, True, "chain const loads")
        add_dep_helper(d2.ins, d3.ins, True, "chain const loads")

        def z4_slice(r, t):
            return z4T_sb[32 * r:32 * r + 32, t * P:(t + 1) * P]

        def cb4_slice(r, c0):
            if c0 < CH0:
                return cb4h_sb[32 * r:32 * r + 32, c0:c0 + 512]
            return cb4t_sb[32 * r:32 * r + 32, c0 - CH0:c0 - CH0 + 512]

        ascale_sb = const_pool.tile([P, NT], DT.float32)
        nc.sync.dma_start(ascale_sb[:], ascale[:])
        zre_sb = const_pool.tile([P, NT * D], DT.float32)
        nc.sync.dma_start(zre_sb[:], zre[:])

        idx_all = const_pool.tile([P, NT], DT.uint32)
        sq_all = const_pool.tile([P, NT], DT.float32)

        for g in range(NG):
            tiles = [g * 4 + r for r in range(4)]
            t16s = [t16_pool.tile([P, K], DT.int16, tag="t16", name=f"t16_g{g}_{r}") for r in range(4)]

            # PE: token tiles stream through disjoint 32-row PE bands.
            # Pair-major order (tiles r,r+1 interleaved per chunk, two
            # shared double-buffered PSUM tags) lets each pair's t16
            # complete halfway through the group, so the DVE max/index
            # stage trails the PE by ~2 tiles instead of a full group.
            if g == 0:
                # Pipeline fill: finish tile 0 first (alternating both PSUM
                # tags) so the DVE max/index stage starts ~15us earlier.
                order = [(ch, r) for r in (0, 1) for ch in range(K // CHUNK)]
                order += [(ch, r) for ch in range(K // CHUNK) for r in (2, 3)]
                tagof = lambda ch, r: (ch if r < 2 else r) % 2
            else:
                order = [(ch, r) for pair in (0, 2) for ch in range(K // CHUNK)
                         for r in (pair, pair + 1)]
                tagof = lambda ch, r: r % 2
            for ch, r in order:
                t = tiles[r]
                ptile = psum_pool.tile([P, CHUNK], DT.float32, tag=f"ps{tagof(ch, r)}", name=f"ps_{g}_{ch}_{r}")
                for j in range(CHUNK // 512):
                    c0 = ch * CHUNK + j * 512
                    nc.tensor.matmul(
                        ptile[:, j * 512:(j + 1) * 512],
                        z4_slice(r, t),
                        cb4_slice(r, c0),
                        start=True,
                        stop=True,
                        tile_position=(32 * r, 0),
                    )

                nc.scalar.activation(
                    t16s[r][:, ch * CHUNK:(ch + 1) * CHUNK],
                    ptile[:],
                    mybir.ActivationFunctionType.Identity,
                    bias=0.0,
                    scale=ascale_sb[:, t:t + 1],
                )

            for r in range(4):
                t = tiles[r]
                t16 = t16s[r]

                # int16 pairwise max tree (tensor_tensor runs 2x on packed
                # 16-bit operands)
                tr1 = tree_pool.tile([P, K // 2], DT.int16)
                nc.vector.tensor_tensor(tr1[:], t16[:, :K // 2], t16[:, K // 2:], op=mybir.AluOpType.max)
                tr2 = tree_pool.tile([P, K // 4], DT.int16)
                nc.vector.tensor_tensor(tr2[:], tr1[:, :K // 4], tr1[:, K // 4:], op=mybir.AluOpType.max)
                tr3 = tree_pool.tile([P, K // 8], DT.int16)
                nc.vector.tensor_tensor(tr3[:], tr2[:, :K // 8], tr2[:, K // 8:], op=mybir.AluOpType.max)
                tr4 = tree_pool.tile([P, K // 16], DT.int16)
                nc.vector.tensor_tensor(tr4[:], tr3[:, :K // 16], tr3[:, K // 16:], op=mybir.AluOpType.max)
                tr5 = tree_pool.tile([P, K // 32], DT.int16)
                nc.vector.tensor_tensor(tr5[:], tr4[:, :K // 32], tr4[:, K // 32:], op=mybir.AluOpType.max)
                gmax8 = small_pool.tile([P, 8], DT.int16, tag="gmax8")
                nc.vector.max(gmax8[:], tr5[:])

                # first index where t16 == gmax (ties resolve to lowest k,
                # matching jnp.argmin); gmax8[:, 0] is the row max.
                idx8 = small_pool.tile([P, 8], DT.uint16, tag="idx8")
                nc.vector.max_index(idx8[:], gmax8[:], t16[:])
                nc.vector.tensor_copy(idx_all[:, t:t + 1], idx8[:, 0:1])

                # z_q gather: codebook row per token
                zq = small_pool.tile([P, D], DT.float32, tag="zq")
                nc.gpsimd.indirect_dma_start(
                    out=zq[:],
                    out_offset=None,
                    in_=cbtab[:],
                    in_offset=bass.IndirectOffsetOnAxis(ap=idx_all[:, t:t + 1], axis=0),
                )

                zslice = zre_sb[:, t * D:(t + 1) * D]
                d1 = small_pool.tile([P, D], DT.float32, tag="d1")
                nc.gpsimd.tensor_tensor(d1[:], zq[:], zslice, op=mybir.AluOpType.subtract)
                zqst = small_pool.tile([P, D], DT.float32, tag="zqst")
                nc.gpsimd.tensor_tensor(zqst[:], zslice, d1[:], op=mybir.AluOpType.add)
                nc.sync.dma_start(zqst_out[:, t * D:(t + 1) * D], zqst[:])

                # loss partial: sum(d1^2) per partition
                junk = small_pool.tile([P, D], DT.float32, tag="junk")
                nc.vector.scalar_tensor_tensor(
                    junk[:],
                    d1[:],
                    1.0,
                    d1[:],
                    op0=mybir.AluOpType.mult,
                    op1=mybir.AluOpType.mult,
                    accum_out=sq_all[:, t:t + 1],
                )

        nc.sync.dma_start(idx_out[:], idx_all[:])
        nc.sync.dma_start(sq_out[:], sq_all[:])

    nc.finalize()
    return nc


def _get_nc() -> bass.Bass:
    global _NC
    if _NC is None:
        _NC = build_nc()
    return _NC


def _prep_core_inputs(z_core: np.ndarray, cb: np.ndarray) -> dict[str, np.ndarray]:
    z_core = np.ascontiguousarray(z_core, dtype=np.float32)   # [T, D]
    A = np.sum(z_core * z_core, axis=1, dtype=np.float32)     # [T]

    z4T = np.tile((2.0 * z_core).T, (4, 1))                   # [128, T]
    cb4T = np.tile(cb.T, (4, 1))                              # [128, K]

    # quantization scale: 1/ulp32(A) = 2^(24 - ex), a power of two.
    _, ex = np.frexp(A.astype(np.float64))
    inv_u = np.exp2(24.0 - ex).astype(np.float32)
    ascale = np.ascontiguousarray(inv_u.reshape(NT, P).T)     # [P, NT]

    zre = np.ascontiguousarray(
        z_core.reshape(NT, P, D).transpose(1, 0, 2).reshape(P, NT * D)
    )
    return {
        "z4T": np.ascontiguousarray(z4T),
        "cb4T": np.ascontiguousarray(cb4T),
        "ascale": ascale,
        "zre": zre,
        "cbtab": np.ascontiguousarray(cb, dtype=np.float32),
    }


def kernel(z: np.ndarray, codebook: np.ndarray):
    z = np.asarray(z, dtype=np.float32)
    cb = np.asarray(codebook, dtype=np.float32)
    assert z.shape == (B, T, D) and cb.shape == (K, D)

    nc = _get_nc()
    in_maps = [_prep_core_inputs(z[i], cb) for i in range(B)]
    trace = bool(int(os.environ.get("VQ_TRACE", "0")))
    res = run_bass_kernel_spmd(nc, in_maps, list(range(B)), trace=trace)
    if trace and res.exec_time_ns is not None:
        print(f"HW exec time: {res.exec_time_ns} ns")

    idx_full = np.empty((B, T), np.int64)
    zqst_full = np.empty((B, T, D), np.float32)
    total_sq = 0.0
    for i in range(B):
        out = res.results[i]
        idx_core = out["idx_out"].astype(np.int64)            # [P, NT]
        idx_full[i] = idx_core.T.reshape(T)                   # token n = t*128+p
        zqst_full[i] = (
            out["zqst_out"].reshape(P, NT, D).transpose(1, 0, 2).reshape(T, D)
        )
        total_sq += float(np.sum(out["sq_out"], dtype=np.float64))

    loss = np.float32(1.25 * total_sq / float(B * T * D))
    idx_out = idx_full.reshape(-1).astype(np.int32)
    return zqst_full, idx_out, loss
